# revision 34
# baseline (speedup 1.0000x reference)
"""Trainium2 Bass kernel for nn_ComplexityDecoderLayer (moe_routing).

Strategy (8 NeuronCores, SPMD, 2 device launches):
  Launch A: attention + INL dynamics.
    - ln1 rmsnorm folded on host (r1 from inputs; ln1_w folded into weights).
    - Per core: 2 q-heads (head-parallel), its GQA kv-head (pairs duplicate).
    - Phase A1 per 512-token chunk: q0/k/v projections + qk-norm/rope +
      head-0 attention (chunks 0,1 also project q1 while their activations
      are resident). The head-0 AllToAll fires right after; phase A2
      (q1 of chunks 3,2 + all head-1 attention) overlaps it, then the
      head-1 AllToAll overlaps phase-3 pass 1.
    - All DRAM operands host-relayouted to [128, ...] per-partition
      contiguous form (large DMA descriptors; naive [(ks p), t] layouts
      produce 512B descriptors at ~58 GB/s). Bulk prefetches serialized
      via explicit deps so the critical first tiles are not starved by
      fair-shared SDMA bandwidth.
    - qk-norm 1/sqrt and softmax 1/sumexp via ScalarE Sqrt +
      DVE reciprocal_approx_fast; partition broadcasts on GpSimd (frees
      PSUM banks, keeps the in-order PE queue free of cross-engine-gated
      tiny matmuls). Norm/rope chains are emitted one projection-group
      late so a slow DVE/ACT step never head-of-line blocks ready
      projection matmuls. Rope tables premultiplied by qk-norm weights.
    - Phase 3 uses host-folded weights Wbig = [Wo | Wo@Wdyn | Wo@Wc1]
      (one GEMM), split by feature-block parity: pass 1 consumes the
      head-0 A2A output while the head-1 A2A is in flight; dynamics fused
      on VectorE. One shared PSUM pool across phases (no pool-boundary
      drain stall).
  Host glue: router top-1 (with fp64 recompute of near-tie tokens), ln2,
    token gather by expert.
  Launch B: balanced expert-parallel MoE: experts paired big-with-small;
    each core = (big expert, quarter-I) + (small expert, quarter-I) with
    token capacities C1/C2 (computed from actual counts + margin; compile
    cached per (C1, C2)). Host sums the four quarter-partials per expert.

Dtypes: fp16 for matmul operands (fp32 PSUM accumulation), bf16 for
attention probabilities (no max-subtraction in softmax; bf16 has the range),
fp16 elementwise where safe; mu stays fp32 (it feeds router logits).
fp8 was evaluated (DoubleRow 1.44x) and rejected: simulated MoE-path
error 1.9-2.4e-2 vs the 2e-2 gate.
"""

import numpy as np

import concourse.bass as bass
import concourse.bacc as bacc
import concourse.tile as tile
from concourse import mybir
from concourse.bass_utils import run_bass_kernel_spmd

# ---- problem constants (hardcoded per spec) ----
T, D, H, KH, HD, I, E, V = 2048, 2048, 16, 4, 128, 5632, 4, 32000
CH, DTC, EPS = 64, 0.1, 1e-6
SCALE = HD ** -0.5
ROPE_BASE = 10000.0
NC = 8                      # cores
TSL = T // NC               # 256 tokens per core
QC = H * HD // NC           # 256 q-cols per core (2 heads)
IH = I // 2                 # 2816 intermediate half per core
CAP = 656                   # MoE per-expert token capacity (max count + margin)
CAPB = CAP - 512            # tail chunk width
KS1 = (2 * D) // 128        # 32 k-subtiles for stacked [x; mu_prev] projections
KSD = D // 128              # 16 k-subtiles over D
KSI = IH // 128             # 22 k-subtiles over IH
NMT = 33                    # phase-3 m-tiles [Wo | Wo@Wdyn | Wo@Wc1 pad]

F32 = mybir.dt.float32
F16 = mybir.dt.float16
BF16 = mybir.dt.bfloat16
AF = mybir.ActivationFunctionType
ALU = mybir.AluOpType

TRACE = False           # set by test.py for profiling
LAST_EXEC_NS = {}       # launch name -> exec ns (when TRACE)

_cache = {}


# ------------------------------------------------------------------ launch A
def _build_launch_a():
    nc = bacc.Bacc("TRN2", target_bir_lowering=False, debug=False, num_devices=NC)

    def din(name, shape, dt):
        return nc.dram_tensor(name, shape, dt, kind="ExternalInput")

    xm = din("xm", [128, 4, KS1, 512], F16)       # [p, chunk, ks, t]
    wst = din("wst", [128, 4, KS1, 128], F16)     # [p, grp(q0,k,v,q1), ks, m]
    wbe = din("wbe", [128, NMT, NC, 128], F16)    # even feature blocks
    wbo = din("wbo", [128, NMT, NC, 128], F16)    # odd feature blocks
    wc2 = din("wc2", [CH, 2 * D], F16)
    bc1 = din("bc1", [CH, 1], F32)
    bc2 = din("bc2", [128, 32], F32)              # bc2[j] at [j % 128, j // 128]
    cq = din("cq", [HD, T], F16)                  # cos * qn
    sq = din("sq", [HD, T], F16)                  # sin_signed * qn_swapped
    ck = din("ck", [HD, T], F16)
    sk = din("sk", [HD, T], F16)
    trimask = din("trimask", [128, 128], BF16)
    ht = din("ht", [128, KSD, TSL], F16)          # raw hidden token slice
    velt = din("velt", [128, KSD, TSL], F16)

    h2t = nc.dram_tensor("h2t", [128, KSD, TSL], F16, kind="ExternalOutput")
    vnt = nc.dram_tensor("vnt", [128, KSD, TSL], F16, kind="ExternalOutput")
    mut = nc.dram_tensor("mut", [128, KSD, TSL], F32, kind="ExternalOutput")

    xm_t = xm.ap()
    wst_t = wst.ap()
    wbe_t = wbe.ap()
    wbo_t = wbo.ap()
    ht_t = ht.ap()
    velt_t = velt.ap()

    with tile.TileContext(nc) as tc:
        with (
            tc.tile_pool(name="dram", bufs=1, space="DRAM") as dram,
            tc.tile_pool(name="const", bufs=1) as const,
            tc.tile_pool(name="psA", bufs=1, space="PSUM") as psA,
        ):
            agin0 = dram.tile([NC, 128, TSL], F16)
            agin1 = dram.tile([NC, 128, TSL], F16)
            agout0 = dram.tile([NC * 128, TSL], F16)
            agout1 = dram.tile([NC * 128, TSL], F16)

            tri_sb = const.tile([128, 128], BF16)
            nc.sync.dma_start(out=tri_sb[:], in_=trimask[:])
            ones_sb = const.tile([128, 1], BF16)   # sumexp lhsT
            nc.vector.memset(ones_sb[:], 1.0)
            ones16 = const.tile([128, 1], F16)     # ssq lhsT
            nc.vector.memset(ones16[:], 1.0)
            onesp_sb = const.tile([1, 128], F32)   # partition-broadcast lhsT
            nc.vector.memset(onesp_sb[:], 1.0)
            eps_sb = const.tile([1, 1], F32)
            nc.vector.memset(eps_sb[:], float(EPS))
            cq_sb = const.tile([HD, T], F16)
            sq_sb = const.tile([HD, T], F16)
            ck_sb = const.tile([HD, T], F16)
            sk_sb = const.tile([HD, T], F16)

            # ------- phases 1+2 interleaved: projections/rope + attention ----
            with (
                tc.tile_pool(name="acts", bufs=1) as acts,
                tc.tile_pool(name="wp", bufs=1) as wp,
                tc.tile_pool(name="xp", bufs=2) as xp,
                tc.tile_pool(name="ep", bufs=6) as ep,
                tc.tile_pool(name="stg", bufs=3) as stg,
            ):
                qr = [acts.tile([128, T], F16, name=f"qr{h}") for h in range(2)]
                kr = acts.tile([128, T], F16)
                vtok = acts.tile([128, KSD, 128], BF16)
                idn = acts.tile([128, 128], BF16)
                nc.gpsimd.memset(idn[:], 0.0)
                from concourse.masks import make_identity
                make_identity(nc, idn[:], nomemset=True)

                from concourse.tile_rust import add_dep_helper
                wall = wp.tile([128, 4, KS1, 128], F16)
                wall_dmas = [nc.sync.dma_start(out=wall[:, g], in_=wst_t[:, g])
                             for g in range(2)]

                def proj_mm(g, xch):
                    # projection GEMM + PSUM evacuation; sqv comes from the
                    # f16 copy so the PSUM slot frees after one DVE op
                    ps = psA.tile([128, 512], F32, tag="pg", bufs=2)
                    for ks in range(KS1):
                        nc.tensor.matmul(
                            ps[:], wall[:, g, ks, :], xch[:, ks, :],
                            start=(ks == 0), stop=(ks == KS1 - 1),
                        )
                    praw = xp.tile([128, 512], F16, tag="praw", bufs=3)
                    nc.vector.tensor_copy(praw[:], ps[:])
                    sqv = xp.tile([128, 512], F16, tag="sqv", bufs=3)
                    nc.vector.tensor_mul(sqv[:], praw[:], praw[:])
                    return praw, sqv

                def norm_rope(praw, sqv, ts, dst, ctab, stab):
                    pssq = psA.tile([1, 512], F32, tag="tiny", bufs=2)
                    nc.tensor.matmul(pssq[:], ones16[:], sqv[:],
                                     start=True, stop=True)
                    srq = xp.tile([1, 512], F32, tag="srq")
                    nc.scalar.activation(srq[:], pssq[:], AF.Sqrt,
                                         bias=eps_sb[:], scale=1.0 / HD)
                    rq = xp.tile([1, 512], F32, tag="rq")
                    nc.vector.reciprocal_approx_fast(out=rq[:], in_=srq[:])
                    pb = xp.tile([128, 512], F32, tag="pb")
                    nc.gpsimd.partition_broadcast(pb[:], rq[:])
                    psw = xp.tile([128, 512], F16, tag="psw")
                    nc.gpsimd.dma_start(out=psw[0:64, :], in_=praw[64:128, :])
                    nc.gpsimd.dma_start(out=psw[64:128, :], in_=praw[0:64, :])
                    a_t = xp.tile([128, 512], F16, tag="ra")
                    nc.vector.tensor_mul(a_t[:], praw[:], ctab[:, ts])
                    b_t = xp.tile([128, 512], F16, tag="rb")
                    nc.vector.tensor_mul(b_t[:], psw[:], stab[:, ts])
                    rr = xp.tile([128, 512], F16, tag="rr")
                    nc.vector.tensor_add(rr[:], a_t[:], b_t[:])
                    nc.vector.tensor_mul(dst[:, ts], rr[:], pb[:])

                def proj_v(nch, xch):
                    ps = psA.tile([128, 512], F32, tag="pg", bufs=2)
                    for ks in range(KS1):
                        nc.tensor.matmul(
                            ps[:], wall[:, 2, ks, :], xch[:, ks, :],
                            start=(ks == 0), stop=(ks == KS1 - 1),
                        )
                    vraw = xp.tile([128, 512], BF16, tag="vraw")
                    nc.vector.tensor_copy(vraw[:], ps[:])
                    for j in range(4):
                        st = 4 * nch + j
                        ptr = psA.tile([128, 128], BF16, tag="sc", bufs=2)
                        nc.tensor.transpose(
                            ptr[:], vraw[:, j * 128:(j + 1) * 128], idn[:])
                        nc.vector.tensor_copy(vtok[:, st, :], ptr[:])

                def att_block(h, tch):
                    t0 = tch * 512
                    nsi = 4 * tch + 4
                    pv = psA.tile([128, 512], F32, tag="pv", bufs=2)
                    se = psA.tile([1, 512], F32, tag="tiny", bufs=2)
                    for si in range(nsi):
                        off = max(si * 128 - t0, 0)
                        sc = psA.tile([128, 512], F32, tag="sc", bufs=2)
                        nc.tensor.matmul(
                            sc[:, off:512],
                            kr[:, si * 128:(si + 1) * 128],
                            qr[h][:, t0 + off:t0 + 512],
                            start=True, stop=True)
                        eb = ep.tile([128, 512], BF16, tag="eb")
                        if si * 128 >= t0:
                            ebd = ep.tile([128, 128], BF16, tag="ebd")
                            nc.scalar.activation(
                                ebd[:], sc[:, off:off + 128],
                                AF.Exp, scale=float(SCALE))
                            nc.vector.tensor_mul(
                                eb[:, off:off + 128], ebd[:], tri_sb[:])
                            if off + 128 < 512:
                                nc.scalar.activation(
                                    eb[:, off + 128:512],
                                    sc[:, off + 128:512],
                                    AF.Exp, scale=float(SCALE))
                        else:
                            nc.scalar.activation(
                                eb[:, off:512], sc[:, off:512],
                                AF.Exp, scale=float(SCALE))
                        nc.tensor.matmul(pv[:, off:512], vtok[:, si, :],
                                         eb[:, off:512],
                                         start=(si == 0),
                                         stop=(si == nsi - 1))
                        nc.tensor.matmul(se[:, off:512], ones_sb[:],
                                         eb[:, off:512],
                                         start=(si == 0),
                                         stop=(si == nsi - 1))
                    rec = stg.tile([1, 512], F32, tag="rec")
                    nc.vector.reciprocal_approx_fast(out=rec[:], in_=se[:])
                    bcs = stg.tile([128, 512], F32, tag="bcs")
                    nc.gpsimd.partition_broadcast(bcs[:], rec[:])
                    att = stg.tile([128, 512], F16, tag="att")
                    nc.vector.tensor_mul(att[:], pv[:], bcs[:])
                    agx = agin0 if h == 0 else agin1
                    for sub in range(2):
                        shard = 2 * tch + sub
                        nc.scalar.dma_start(
                            out=agx[shard, :, :],
                            in_=att[:, sub * 256:(sub + 1) * 256])

                # phase A1: q0/k/v projections + head-0 attention per chunk
                # (chunks 0,1 also run q1 here while their xch is resident),
                # so the head-0 AllToAll fires as early as possible and
                # phase A2 runs entirely from resident data.
                xcs = {}
                prev_dma = None
                for nch in range(T // 512):
                    ts = slice(nch * 512, nch * 512 + 512)
                    xch = xp.tile([128, KS1, 512], F16, tag="xch")
                    xcs[nch] = xch
                    if nch == 0:
                        for qq in range(4):
                            dq = nc.gpsimd.dma_start(
                                out=xch[:, 8 * qq:8 * qq + 8, :],
                                in_=xm_t[:, 0, 8 * qq:8 * qq + 8, :])
                            if qq == 1:
                                d0 = dq
                        # defer non-critical weight halves behind chunk 0
                        wd2_ = nc.sync.dma_start(out=wall[:, 2],
                                                 in_=wst_t[:, 2])
                        wd3_ = nc.sync.dma_start(out=wall[:, 3],
                                                 in_=wst_t[:, 3])
                        add_dep_helper(wd2_.ins, d0.ins, True, "serialize")
                        add_dep_helper(wd3_.ins, d0.ins, True, "serialize")
                        prev_dma = dq
                    else:
                        # sync queue: keeps bulk prefetch off the gpsimd
                        # queue (whose dma-starts sit behind rope broadcasts)
                        dx = nc.sync.dma_start(out=xch[:], in_=xm_t[:, nch])
                        add_dep_helper(dx.ins, prev_dma.ins, True, "serialize")
                        prev_dma = dx
                    for tab_sb, tab in ((cq_sb, cq), (sq_sb, sq),
                                        (ck_sb, ck), (sk_sb, sk)):
                        nc.scalar.dma_start(out=tab_sb[:, ts],
                                            in_=tab.ap()[:, ts])
                    # lagged emission: group g's norm chain is emitted after
                    # group g+1's GEMM, so a slow DVE/ACT step never blocks
                    # the in-order PE queue ahead of ready projection MMs
                    pq0 = proj_mm(0, xch)
                    pk = proj_mm(1, xch)
                    norm_rope(*pq0, ts, qr[0], cq_sb, sq_sb)
                    proj_v(nch, xch)
                    norm_rope(*pk, ts, kr, ck_sb, sk_sb)
                    att_block(0, nch)
                    if nch < 2:
                        pq1 = proj_mm(3, xch)
                        norm_rope(*pq1, ts, qr[1], cq_sb, sq_sb)
                nc.gpsimd.collective_compute(
                    "AllToAll", ALU.bypass,
                    replica_groups=[list(range(NC))],
                    ins=[agin0[:].rearrange("a b c -> (a b) c")],
                    outs=[agout0[:]],
                )
                # phase A2: q1 of chunks 3,2 (xch still resident; bufs=2) +
                # head-1 attention overlap the head-0 AllToAll.
                pq3 = proj_mm(3, xcs[3])
                pq2 = proj_mm(3, xcs[2])
                norm_rope(*pq3, slice(3 * 512, 4 * 512), qr[1], cq_sb, sq_sb)
                norm_rope(*pq2, slice(2 * 512, 3 * 512), qr[1], cq_sb, sq_sb)
                for nch in (3, 2, 1, 0):
                    att_block(1, nch)

            nc.gpsimd.collective_compute(
                "AllToAll", ALU.bypass,
                replica_groups=[list(range(NC))],
                ins=[agin1[:].rearrange("a b c -> (a b) c")],
                outs=[agout1[:]],
            )

            # ---- phase 3: fused [o-proj | mu | ctrl-l1] GEMM + dynamics -----
            # Y[:, mt] for mt<16: attnO^T; 16..31: mu^T; 32: pre-tanh ctrl.
            ag0_t = agout0[:].rearrange("(ks p) t -> p ks t", p=128)
            ag1_t = agout1[:].rearrange("(ks p) t -> p ks t", p=128)
            with (
                tc.tile_pool(name="agp", bufs=1) as agp,
                tc.tile_pool(name="w3", bufs=6) as w3,
                tc.tile_pool(name="o3", bufs=1) as o3,
                tc.tile_pool(name="dy", bufs=3) as dy,
            ):
                agk0 = agp.tile([128, NC, TSL], F16)
                for j in range(NC):
                    eng = nc.scalar if j % 2 == 0 else nc.sync
                    eng.dma_start(out=agk0[:, j, :], in_=ag0_t[:, j, :])
                agk1 = agp.tile([128, NC, TSL], F16)
                for j in range(NC):
                    eng = nc.scalar if j % 2 == 0 else nc.sync
                    eng.dma_start(out=agk1[:, j, :], in_=ag1_t[:, j, :])
                velsb = o3.tile([128, KSD, TSL], F16)
                nc.scalar.dma_start(out=velsb[:], in_=velt_t[:])
                htsb = o3.tile([128, KSD, TSL], F16)
                nc.scalar.dma_start(out=htsb[:], in_=ht_t[:])
                Y = o3.tile([128, NMT, TSL], F32)
                ctl = o3.tile([128, 32, TSL], F16)        # [alpha; beta]^T
                bc1_sb = o3.tile([CH, 1], F32)
                nc.scalar.dma_start(out=bc1_sb[:], in_=bc1.ap())
                bc2_sb = o3.tile([128, 32], F32)
                nc.scalar.dma_start(out=bc2_sb[:], in_=bc2.ap())
                wc2_sb = o3.tile([CH, 2 * D], F16)
                nc.scalar.dma_start(out=wc2_sb[:], in_=wc2.ap())

                # pass 1: even feature blocks (head-0 A2A output)
                for mt in range(NMT):
                    wt = w3.tile([128, NC, 128], F16, tag="we")
                    nc.sync.dma_start(out=wt[:], in_=wbe_t[:, mt])
                    po = psA.tile([128, TSL], F32, tag="pg", bufs=2)
                    for ks in range(NC):
                        nc.tensor.matmul(po[:], wt[:, ks, :], agk0[:, ks, :],
                                         start=(ks == 0), stop=(ks == NC - 1))
                    nc.vector.tensor_copy(Y[:, mt, :], po[:])

                # pass 2: odd feature blocks; mt=32 (ctrl) first so the
                # controller chain overlaps the remaining matmuls.
                def pass2(mt):
                    wt = w3.tile([128, NC, 128], F16, tag="wo")
                    nc.sync.dma_start(out=wt[:], in_=wbo_t[:, mt])
                    po = psA.tile([128, TSL], F32, tag="pg", bufs=2)
                    for ks in range(NC):
                        nc.tensor.matmul(po[:], wt[:, ks, :], agk1[:, ks, :],
                                         start=(ks == 0), stop=(ks == NC - 1))
                    nc.vector.tensor_add(Y[:, mt, :], Y[:, mt, :], po[:])

                pass2(32)
                tanh_sb = o3.tile([CH, TSL], F16)
                nc.scalar.activation(tanh_sb[:], Y[0:CH, 32, :], AF.Tanh,
                                     bias=bc1_sb[:])
                for mt in range(32):
                    pc2 = psA.tile([128, TSL], F32, tag="sc", bufs=2)
                    nc.tensor.matmul(pc2[:], wc2_sb[:, mt * 128:(mt + 1) * 128],
                                     tanh_sb[:], start=True, stop=True)
                    nc.scalar.activation(ctl[:, mt, :], pc2[:], AF.Sigmoid,
                                         bias=bc2_sb[:, mt:mt + 1])

                # remaining pass-2 pairs (mu, attnO) + dynamics per mt.
                # v_new = vel + DT*(alpha*(mu-attn) - beta*vel)
                for mt in range(KSD):
                    pass2(16 + mt)
                    pass2(mt)
                    nc.scalar.dma_start(out=mut.ap()[:, mt, :],
                                        in_=Y[:, 16 + mt, :])
                    d_t = dy.tile([128, TSL], F16, tag="d")
                    nc.vector.tensor_sub(d_t[:], Y[:, 16 + mt, :], Y[:, mt, :])
                    t2 = dy.tile([128, TSL], F16, tag="t2")
                    nc.vector.tensor_mul(t2[:], ctl[:, mt, :], d_t[:])
                    t3 = dy.tile([128, TSL], F16, tag="t3")
                    nc.vector.tensor_mul(t3[:], ctl[:, mt + 16, :],
                                         velsb[:, mt, :])
                    g_t = dy.tile([128, TSL], F16, tag="g")
                    nc.vector.tensor_sub(g_t[:], t2[:], t3[:])
                    vn_t = dy.tile([128, TSL], F16, tag="vn")
                    nc.vector.scalar_tensor_tensor(
                        vn_t[:], g_t[:], float(DTC), velsb[:, mt, :],
                        op0=ALU.mult, op1=ALU.add)
                    nc.scalar.dma_start(out=vnt.ap()[:, mt, :], in_=vn_t[:])
                    hd_t = dy.tile([128, TSL], F16, tag="hd")
                    nc.vector.scalar_tensor_tensor(
                        hd_t[:], vn_t[:], float(DTC), Y[:, mt, :],
                        op0=ALU.mult, op1=ALU.add)
                    hd2 = dy.tile([128, TSL], F16, tag="hd2")
                    nc.vector.tensor_add(hd2[:], hd_t[:], htsb[:, mt, :])
                    nc.scalar.dma_start(out=h2t.ap()[:, mt, :], in_=hd2[:])

    nc.compile()
    return nc


# ------------------------------------------------------------------ launch B
# Balanced expert-parallel MoE: experts paired big-with-small; each core
# handles (big expert, quarter-I) + (small expert, quarter-I). The four
# quarter-partials per expert are summed on the host. Capacities C1/C2
# cover the largest big/small expert (token counts padded).
MQ = KSI // 2  # 11 I-mtiles per quarter


def _chunks(CX):
    return [(c0, min(512, CX - c0)) for c0 in range(0, CX, 512)]


def _build_launch_b(C1, C2):
    nc = bacc.Bacc("TRN2", target_bir_lowering=False, debug=False, num_devices=NC)

    slots = []
    for si, CX in ((0, C1), (1, C2)):
        x2 = nc.dram_tensor(f"x2_{si}", [128, KSD, CX], F16,
                            kind="ExternalInput")
        wg = nc.dram_tensor(f"wg{si}", [128, MQ, KSD, 128], F16,
                            kind="ExternalInput")
        wu = nc.dram_tensor(f"wu{si}", [128, MQ, KSD, 128], F16,
                            kind="ExternalInput")
        wd = nc.dram_tensor(f"wd{si}", [128, KSD, MQ, 128], F16,
                            kind="ExternalInput")
        het = nc.dram_tensor(f"het{si}", [128, KSD, CX], F16,
                             kind="ExternalOutput")
        slots.append((si, CX, x2.ap(), wg.ap(), wu.ap(), wd.ap(), het.ap()))

    with tile.TileContext(nc) as tc:
        with (
            tc.tile_pool(name="xc", bufs=2) as xc,
            tc.tile_pool(name="wp", bufs=3) as wp,
            tc.tile_pool(name="ac", bufs=2) as ac,
            tc.tile_pool(name="st", bufs=3) as st,
            tc.tile_pool(name="ps", bufs=2, space="PSUM") as ps,
        ):
            for si, CX, x2_t, wg_t, wu_t, wd_t, het_t in slots:
                chunks = _chunks(CX)
                x2s = xc.tile([128, KSD, C1], F16, tag="x2")
                for q in range(4):
                    nc.scalar.dma_start(out=x2s[:, 4 * q:4 * q + 4, :CX],
                                        in_=x2_t[:, 4 * q:4 * q + 4, :])
                act = ac.tile([128, MQ, C1], F16, tag="act")

                for mt in range(MQ):
                    wgm = wp.tile([128, KSD, 128], F16, tag="wg")
                    nc.sync.dma_start(out=wgm[:], in_=wg_t[:, mt])
                    wum = wp.tile([128, KSD, 128], F16, tag="wu")
                    nc.sync.dma_start(out=wum[:], in_=wu_t[:, mt])
                    for c0, cw in chunks:
                        pg = ps.tile([128, 512], F32, tag="pg")
                        pu = ps.tile([128, 512], F32, tag="pu")
                        for ks in range(KSD):
                            nc.tensor.matmul(
                                pg[:, :cw], wgm[:, ks, :],
                                x2s[:, ks, c0:c0 + cw],
                                start=(ks == 0), stop=(ks == KSD - 1))
                        for ks in range(KSD):
                            nc.tensor.matmul(
                                pu[:, :cw], wum[:, ks, :],
                                x2s[:, ks, c0:c0 + cw],
                                start=(ks == 0), stop=(ks == KSD - 1))
                        sg = st.tile([128, 512], F32, tag="sg")
                        nc.scalar.activation(sg[:, :cw], pg[:, :cw], AF.Silu)
                        nc.vector.tensor_mul(act[:, mt, c0:c0 + cw],
                                             sg[:, :cw], pu[:, :cw])

                for mt in range(KSD):
                    wdm = wp.tile([128, MQ, 128], F16, tag="wd")
                    nc.sync.dma_start(out=wdm[:], in_=wd_t[:, mt])
                    for c0, cw in chunks:
                        pd = ps.tile([128, 512], F32, tag="pd")
                        for ks in range(MQ):
                            nc.tensor.matmul(
                                pd[:, :cw], wdm[:, ks, :],
                                act[:, ks, c0:c0 + cw],
                                start=(ks == 0), stop=(ks == MQ - 1))
                        ot = st.tile([128, 512], F16, tag="ot")
                        nc.vector.tensor_copy(ot[:, :cw], pd[:, :cw])
                        nc.sync.dma_start(out=het_t[:, mt, c0:c0 + cw],
                                          in_=ot[:, :cw])

    nc.compile()
    return nc


# ------------------------------------------------------------------ host glue
def _rope_tables(positions):
    inv = 1.0 / (ROPE_BASE ** (np.arange(0, HD, 2, dtype=np.float64) / HD))
    ang = positions.astype(np.float64)[:, None] * inv[None, :]     # [T, 64]
    cosT = np.cos(ang).T.astype(np.float32)                        # [64, T]
    sinT = np.sin(ang).T.astype(np.float32)
    cosf = np.concatenate([cosT, cosT], axis=0)                    # [128, T]
    sins = np.concatenate([-sinT, sinT], axis=0)
    return cosf, sins


LAST_RES = {}


def _run(nc_obj, in_maps, name):
    res = run_bass_kernel_spmd(nc_obj, in_maps, list(range(NC)), trace=TRACE)
    if TRACE:
        LAST_EXEC_NS[name] = res.exec_time_ns
        LAST_RES[name] = res
    return res.results


def _np_softmax(x, axis=-1):
    m = x.max(axis=axis, keepdims=True)
    e = np.exp(x - m)
    return e / e.sum(axis=axis, keepdims=True)


def _p128(a, ks_dim):
    """[ks*128+p, ...free] -> [128, ks, ...free] contiguous fp16."""
    s = a.shape
    r = a.reshape(ks_dim, 128, *s[1:])
    order = (1, 0) + tuple(range(2, r.ndim))
    return np.ascontiguousarray(r.transpose(order))


def _exact_mu_rows(inputs, risk_idx):
    """Reference-faithful fp64 recompute of mu rows for near-tie tokens."""
    f = np.float64
    hidden = inputs["hidden_states"].astype(f)
    mu_prev = inputs["mu_prev"].astype(f)
    pos = np.asarray(inputs["positions"]).astype(np.int64)
    x = hidden * (1.0 / np.sqrt((hidden ** 2).mean(-1, keepdims=True) + EPS))
    x = x * inputs["ln1_w"].astype(f)[None, :]
    k = x @ inputs["Wk"].astype(f) + mu_prev @ inputs["Wmu_k"].astype(f)
    v = x @ inputs["Wv"].astype(f) + mu_prev @ inputs["Wmu_v"].astype(f)
    k = k.reshape(T, KH, HD)
    v = v.reshape(T, KH, HD)
    k = k * (1.0 / np.sqrt((k ** 2).mean(-1, keepdims=True) + EPS))
    k = k * inputs["kn_w"].astype(f)

    inv = 1.0 / (ROPE_BASE ** (np.arange(0, HD, 2, dtype=f) / HD))
    ang = pos[:, None] * inv[None, :]
    cos, sin = np.cos(ang)[:, None, :], np.sin(ang)[:, None, :]

    def rope(t):
        t1, t2 = t[..., :64], t[..., 64:]
        return np.concatenate([t1 * cos - t2 * sin, t2 * cos + t1 * sin], -1)

    k = rope(k)
    Wq = inputs["Wq"].astype(f)
    Wmq = inputs["Wmu_q"].astype(f)
    Wo = inputs["Wo"].astype(f)
    Wdyn = inputs["Wdyn_mu"].astype(f)
    qn_w = inputs["qn_w"].astype(f)
    mu_rows = np.zeros((len(risk_idx), D), np.float64)
    rep = H // KH
    for n, t in enumerate(risk_idx):
        q = x[t] @ Wq + mu_prev[t] @ Wmq
        q = q.reshape(H, HD)
        q = q * (1.0 / np.sqrt((q ** 2).mean(-1, keepdims=True) + EPS)) * qn_w
        q1, q2 = q[:, :64], q[:, 64:]
        c, s = np.cos(ang[t]), np.sin(ang[t])
        q = np.concatenate([q1 * c - q2 * s, q2 * c + q1 * s], -1)    # [H, HD]
        kk = np.repeat(k[: t + 1], rep, axis=1)                      # [t+1, H, HD]
        vv = np.repeat(v[: t + 1], rep, axis=1)
        sc = np.einsum("hd,shd->hs", q, kk) * SCALE
        pr = _np_softmax(sc, axis=-1)
        at = np.einsum("hs,shd->hd", pr, vv).reshape(H * HD)
        mu_rows[n] = (at @ Wo) @ Wdyn
    return mu_rows


def kernel(**inputs):
    f32 = np.float32
    f16 = np.float16
    hidden = np.ascontiguousarray(inputs["hidden_states"], dtype=f32)
    vel = np.ascontiguousarray(inputs["velocity_states"], dtype=f32)
    mu_prev = np.ascontiguousarray(inputs["mu_prev"], dtype=f32)
    pos = np.asarray(inputs["positions"]).astype(np.int64)
    tok = np.asarray(inputs["token_ids"]).astype(np.int64)
    ln1 = np.asarray(inputs["ln1_w"], dtype=f32)
    ln2 = np.asarray(inputs["ln2_w"], dtype=f32)

    # ---- host prep for launch A ----
    r1 = 1.0 / np.sqrt((hidden.astype(np.float64) ** 2).mean(-1) + EPS)
    xT = (hidden * r1[:, None].astype(f32) * ln1[None, :]).T
    xmT = np.concatenate([xT, mu_prev.T], axis=0).astype(f16)  # [2D, T]
    # [128, chunk, ks, t]
    xm3 = np.ascontiguousarray(
        xmT.reshape(KS1, 128, 4, 512).transpose(1, 2, 0, 3))

    cosf, sins = _rope_tables(pos)
    trimask = np.triu(np.ones((128, 128), np.float32))  # [s, t]: t >= s
    import ml_dtypes
    trimask = trimask.astype(ml_dtypes.bfloat16)

    qn = np.asarray(inputs["qn_w"], f32).reshape(HD, 1)
    kn = np.asarray(inputs["kn_w"], f32).reshape(HD, 1)
    qnsw = np.concatenate([qn[64:], qn[:64]], axis=0)
    knsw = np.concatenate([kn[64:], kn[:64]], axis=0)
    cq_h = (cosf * qn).astype(f16)
    sq_h = (sins * qnsw).astype(f16)
    ck_h = (cosf * kn).astype(f16)
    sk_h = (sins * knsw).astype(f16)

    Wq = np.asarray(inputs["Wq"], f32)
    Wmq = np.asarray(inputs["Wmu_q"], f32)
    Wk = np.asarray(inputs["Wk"], f32)
    Wmk = np.asarray(inputs["Wmu_k"], f32)
    Wv = np.asarray(inputs["Wv"], f32)
    Wmv = np.asarray(inputs["Wmu_v"], f32)
    Wof = np.asarray(inputs["Wo"], f32)
    Wdynf = np.asarray(inputs["Wdyn_mu"], f32)
    Wc1f = np.asarray(inputs["Wc1"], f32)
    # fused phase-3 weights: [Wo | Wo@Wdyn | Wo@Wc1 | 0], split by
    # feature-block parity to match the per-head A2A arrival order
    Wbig = np.concatenate(
        [Wof, Wof @ Wdynf, Wof @ Wc1f, np.zeros((D, 64), f32)], axis=1)
    Wr = Wbig.reshape(D // 128, 128, NMT * 128)
    wbe_h = np.ascontiguousarray(
        Wr[0::2].reshape(D // 2, NMT * 128)).astype(f16)
    wbo_h = np.ascontiguousarray(
        Wr[1::2].reshape(D // 2, NMT * 128)).astype(f16)
    # [128, mt, j, m]
    wbe2 = np.ascontiguousarray(
        wbe_h.reshape(NC, 128, NMT, 128).transpose(1, 2, 0, 3))
    wbo2 = np.ascontiguousarray(
        wbo_h.reshape(NC, 128, NMT, 128).transpose(1, 2, 0, 3))
    Wc2 = np.asarray(inputs["Wc2"], f32).astype(f16)
    bc1 = np.asarray(inputs["bc1"], f32).reshape(CH, 1)
    bc2 = np.asarray(inputs["bc2"], f32).reshape(32, 128).T.copy()  # [128, 32]

    if "A" not in _cache:
        _cache["A"] = _build_launch_a()
    in_maps = []
    for c in range(NC):
        g = c // 2
        qs = slice(c * QC, (c + 1) * QC)
        ks_ = slice(g * HD, (g + 1) * HD)
        wst = np.concatenate([
            np.concatenate([Wq[:, qs], Wmq[:, qs]], axis=0),
            np.concatenate([Wk[:, ks_], Wmk[:, ks_]], axis=0),
            np.concatenate([Wv[:, ks_], Wmv[:, ks_]], axis=0),
        ], axis=1).astype(f16)  # [2D, 512] as [q0|q1|k|v]
        # execution-order columns [q0 | k | v | q1]
        wst = np.ascontiguousarray(
            wst[:, np.r_[0:128, 256:384, 384:512, 128:256]])
        # [128, grp, ks, m]
        wst2 = np.ascontiguousarray(
            wst.reshape(KS1, 128, 4, 128).transpose(1, 2, 0, 3))
        tsl = slice(c * TSL, (c + 1) * TSL)
        ht2 = np.ascontiguousarray(
            hidden[tsl].T.reshape(KSD, 128, TSL).transpose(1, 0, 2)
        ).astype(f16)
        velt2 = np.ascontiguousarray(
            vel[tsl].T.reshape(KSD, 128, TSL).transpose(1, 0, 2)
        ).astype(f16)
        in_maps.append({
            "xm": xm3, "wst": wst2, "wbe": wbe2, "wbo": wbo2,
            "wc2": Wc2, "bc1": bc1, "bc2": bc2,
            "cq": cq_h, "sq": sq_h, "ck": ck_h, "sk": sk_h,
            "trimask": trimask,
            "ht": ht2, "velt": velt2,
        })
    res_a = _run(_cache["A"], in_maps, "A")

    def _unp(a):  # [128, ks, t] -> [ks*128, t]
        return a.transpose(1, 0, 2).reshape(D, TSL)

    hidden2 = np.concatenate(
        [_unp(res_a[c]["h2t"]) for c in range(NC)], axis=1).T
    v_new = np.concatenate(
        [_unp(res_a[c]["vnt"]) for c in range(NC)], axis=1).T
    mu = np.concatenate(
        [_unp(res_a[c]["mut"]) for c in range(NC)], axis=1).T
    hidden2 = np.ascontiguousarray(hidden2, f32)
    v_new = np.ascontiguousarray(v_new, f32)
    mu = np.ascontiguousarray(mu, f32)

    # ---- routing on host (fp64; near-tie tokens recomputed exactly) ----
    rt = np.asarray(inputs["router_table"], f32)[tok]              # [T, E]
    Wmur = np.asarray(inputs["Wmu_router"], f32)
    logits = rt.astype(np.float64) + mu.astype(np.float64) @ Wmur.astype(np.float64)
    srt = np.sort(logits, axis=-1)
    risk = np.where(srt[:, -1] - srt[:, -2] < 2e-3)[0]
    if len(risk) > 0:
        mu_fix = _exact_mu_rows(inputs, risk)
        logits[risk] = rt[risk].astype(np.float64) + mu_fix @ Wmur.astype(np.float64)
    eidx = logits.argmax(-1)
    gate = _np_softmax(logits, axis=-1)[np.arange(T), eidx].astype(f32)

    # ---- launch B: gathered expert FFN ----
    r2 = 1.0 / np.sqrt((hidden2.astype(np.float64) ** 2).mean(-1) + EPS)
    x2 = hidden2 * r2[:, None].astype(f32) * ln2[None, :]
    Wg = np.asarray(inputs["Wg"], f32)
    Wu = np.asarray(inputs["Wu"], f32)
    Wd = np.asarray(inputs["Wd"], f32)

    idx_e = [np.where(eidx == e)[0] for e in range(E)]
    counts = [len(ix) for ix in idx_e]
    # pair the biggest expert with the smallest: per-core work is then
    # (big + small) ~ balanced; C1/C2 are the slot capacities (+margin
    # for near-tie routing flips across runs)
    order = np.argsort(counts)[::-1]
    pairs = [(int(order[0]), int(order[3])), (int(order[1]), int(order[2]))]

    def pad16(n):
        return max(16, int(np.ceil((n + 8) / 16) * 16))

    C1n = pad16(max(counts[pairs[0][0]], counts[pairs[1][0]]))
    C2n = pad16(max(counts[pairs[0][1]], counts[pairs[1][1]]))
    key = ("B", C1n, C2n)
    if key not in _cache:
        _cache[key] = _build_launch_b(C1n, C2n)

    in_maps_b = []
    for c in range(NC):
        p, q = c // 4, c % 4
        qs = slice(q * MQ * 128, (q + 1) * MQ * 128)
        mm = {}
        for si in range(2):
            e = pairs[p][si]
            CX = C1n if si == 0 else C2n
            x2gT = np.zeros((D, CX), f16)
            x2gT[:, :counts[e]] = x2[idx_e[e]].T
            mm[f"x2_{si}"] = _p128(x2gT, KSD)
            mm[f"wg{si}"] = np.ascontiguousarray(
                np.asarray(Wg[e][:, qs], f16)
                .reshape(KSD, 128, MQ, 128).transpose(1, 2, 0, 3))
            mm[f"wu{si}"] = np.ascontiguousarray(
                np.asarray(Wu[e][:, qs], f16)
                .reshape(KSD, 128, MQ, 128).transpose(1, 2, 0, 3))
            mm[f"wd{si}"] = np.ascontiguousarray(
                np.asarray(Wd[e][qs, :], f16)
                .reshape(MQ, 128, KSD, 128).transpose(1, 2, 0, 3))
        in_maps_b.append(mm)
    res_b = _run(_cache[key], in_maps_b, "B")

    out = hidden2.copy()
    for p in range(2):
        for si in range(2):
            e = pairs[p][si]
            n = counts[e]
            if n == 0:
                continue
            CX = C1n if si == 0 else C2n
            acc = np.zeros((D, n), np.float32)
            for c in range(4 * p, 4 * p + 4):
                acc += (res_b[c][f"het{si}"].transpose(1, 0, 2)
                        .reshape(D, CX)[:, :n].astype(np.float32))
            out[idx_e[e]] += gate[idx_e[e], None] * acc.T

    return out, v_new, mu
